# revision 24
# baseline (speedup 1.0000x reference)
"""Trainium2 Bass kernel for nn_Attention_35708358099413.

Reference computation (T=8192, B=64, H=256, N=128):
    sW     = s_before @ W.T + b                      # [1,B,H]
    denom  = einsum('obd,tbd->ob', sW, h)            # [1,B] (sum over T and H)
    scores = einsum('obd,nbd->obn', sW, h_sliced) / denom
    c_t    = (scores.T * h_sliced).sum(0)            # [B,H]

Strategy: pure data-parallel over batch. 8 cores x 8 batches each; no
collectives. Per core the dominant work is h_sum[b,d] = sum_t h[t,b,d]
— a pure streaming reduction, so HBM read bytes are the binding
resource end to end.

KEY LEVER — h streams as fp16 (host-side cast in _shard_inputs):
32MB/core instead of 64MB. This is safe because h only enters the
output through denom[b] = <sW[b], h_sum[b]>: the quantization error is
MULTIPLICATIVE per batch (~5e-4 for fp16), nothing additive hits the
near-zero output elements. bf16 would NOT pass: the observed max-rel
comes from a batch with a small denominator, and scaling fp16's 3.5e-3
max-rel by the 8x eps ratio lands over the 2e-2 gate. The scores/c_t
path (h_sliced, s, W, b) stays fp32: any quantization there adds
ADDITIVE noise that blows up max-rel on near-zero outputs.

Under full 8-core overlap each core sustains ~317-330 GB/s of HBM
read (the ~358 GB/s HBM-per-NC share minus arbitration); measured
chunk-size sweep: 1MB chunks (8KB/partition contiguous descriptors,
cr=2) best; 2MB equal; 4MB and 0.5MB worse. h streams as 31 full
chunks alternating the two HWDGE rings (sync 16 / scalar 15) + the
last chunk as two 0.5MB [128, 2048] sub-chunks both on scalar (evens
the rings at ~16.65MB each AND halves the final matmul lag after the
last byte). The ~1.3MB of fp32 small loads are ring-balanced: s/b +
W-half + hs[0:64] on sync, W-half + hs[64:128] on scalar. Do NOT
split chunks into strided sub-1MB DMAs (4KB strided descriptors
collapse to a single-SDMA-engine trickle); the tail sub-chunks are
whole contiguous [128, F] blocks, which are fine.

Reduction over T on the TensorEngine as fp16 matmuls: lhsT =
e3[:, b, :] (ones in column b) lands batch b's column sums on PSUM
partition b. Chunks 0..30 accumulate into ps8; the last chunk into a
separate ps8b, so the <sW, ps8> denominator partial runs on DVE while
the last chunk is still streaming — after the final matmul only the
small ps8b partial + combine + reciprocal + scale + 8KB store remain
(~2.5us tail). Matmuls are emitted per-chunk (16 per 1MB chunk) so
the in-order PE tracks DMA arrivals; warm PE does N=256 matmuls in
~109ns, well ahead of the ~2.9us/chunk DMA cadence.

sW = s @ W.T + b on PE from on-chip transposes of s and W, emitted
after chunk 2; sW is broadcast to all 128 partitions by placing it
block-diagonally and multiplying by ones8 on PE — no DRAM bounce.
scores_raw[n,b] = rowwise reduce of (h_sliced * bcast_sW) on DVE;
c_raw[b,:] = scores^T @ h_sliced on PE via masked score columns.

Measured: 220us (f32r, 64MB) -> ~111us (fp16, 32MB) on 8-core SPMD;
breakdown ~7.7us NRT+Tile preamble, ~93-101us stream at the HBM wall,
~2.5us tail compute+store, ~8.7us NRT postamble + profile flush (the
pre/postamble are kernel-invariant: 51 sems/engine reset, barriers).
Rep jitter is bimodal (+0/+10us) from cross-core HBM contention phase.
"""

import json

import numpy as np

T, B, H, N = 8192, 64, 256, 128
NCORES = 8
BL = B // NCORES          # 8 batches per core
F = BL * H                # 2048

_CACHE = {}


def _split_multi_waits(bir_bytes, max_waits=1):
    """Walrus in some containers rejects instructions carrying more than
    one sem wait ("Too many sync wait commands"). Move excess waits onto
    preceding same-engine Drain carrier instructions."""
    m = json.loads(bir_bytes)
    for fn in m.get("functions", []):
        for bb in fn.get("blocks", []):
            out = []
            for inst in bb.get("instructions", []):
                si = inst.get("sync_info") or {}
                w = si.get("on_wait") or []
                if len(w) > max_waits:
                    head = w[: len(w) - max_waits]
                    si["on_wait"] = w[len(w) - max_waits:]
                    inst["sync_info"] = si
                    for k, wt in enumerate(head):
                        out.append({
                            "name": f"{inst['name']}_wsplit{k}",
                            "engine": inst["engine"],
                            "opcode": "Drain",
                            "ins": [], "outs": [],
                            "is_reset_sema": False,
                            "debug": inst.get("debug"),
                            "sync_info": {"on_wait": [wt], "on_update": []},
                        })
                out.append(inst)
            bb["instructions"] = out
    return json.dumps(m).encode()


def _install_birpatch(nc):
    orig = nc.to_json_bytes
    nc.to_json_bytes = lambda: _split_multi_waits(orig())


def _build(t_total=T, cr=2, hbufs=20, hdt_name="f16",
           sw_at=-1, sc1_at=-1, sc2_at=-1):
    """cr = h-rows per partition per chunk (chunk contiguous; for f16
    cr=2 keeps the proven 8KB/partition descriptor shape = 1MB chunk).
    hdt_name: dtype h is streamed in. 'f16' halves HBM bytes vs f32/f32r;
    the h path only perturbs each batch's denominator scale
    (multiplicative, ~5e-4 for fp16) so the rel-err gate is safe."""
    import concourse.bass as bass
    import concourse.mybir as mybir
    from concourse import tile

    f32 = mybir.dt.float32
    X = mybir.AxisListType.X
    AO = mybir.AluOpType

    rows_per_chunk = 128 * cr
    nch = t_total // rows_per_chunk
    assert nch * rows_per_chunk == t_total
    # The PE executes strictly in order: anything the sW/scores path
    # waits on stalls every later h-chunk matmul behind it. Place the
    # stages so their DVE/DMA dependencies are long since ready when the
    # PE reaches them: sW matmuls after chunk 4 (s/W/b land ~chunk 3),
    # scores_part1 after chunk 6 (hs lands ~chunk 5), scores_part2 after
    # chunk 20 (the ~5us DVE prod/reduce chain has ~14 chunks of slack).
    if sw_at < 0:
        sw_at = min(4, nch - 4)
    if sc1_at < 0:
        sc1_at = min(6, nch - 3)
    if sc2_at < 0:
        sc2_at = (5 * nch) // 8

    hdt = {"f16": mybir.dt.float16, "bf16": mybir.dt.bfloat16,
           "f32r": mybir.dt.float32r, "f32": f32}[hdt_name]

    nc = bass.Bass()
    h_d = nc.dram_tensor("h", [t_total, F], hdt, kind="ExternalInput")
    hs_d = nc.dram_tensor("hs", [N, F], f32, kind="ExternalInput")
    # s and W arrive HOST-TRANSPOSED (d-major): sW = s @ W.T contracts
    # over d, and the PE contracts over partitions — with d on the
    # partition dim both matmul operands come straight from DRAM, no
    # on-chip transposes (the old PE-transpose -> DVE-copy ping-pong
    # stalled every later h-chunk matmul behind it in program order).
    st_d = nc.dram_tensor("st", [H, BL], f32, kind="ExternalInput")
    wt_d = nc.dram_tensor("wt", [H, H], f32, kind="ExternalInput")
    b_d = nc.dram_tensor("bias", [1, H], f32, kind="ExternalInput")
    out_d = nc.dram_tensor("out", [BL, H], f32, kind="ExternalOutput")

    with tile.TileContext(nc) as tc:
        with (
            tc.tile_pool(name="consts", bufs=1) as consts,
            tc.tile_pool(name="small", bufs=1) as small,
            tc.tile_pool(name="hpool", bufs=hbufs) as hpool,
            tc.tile_pool(name="psum", bufs=1, space=bass.MemorySpace.PSUM) as psum,
            tc.tile_pool(name="psumb", bufs=1, space=bass.MemorySpace.PSUM) as psumb,
        ):
            # ---- first chunk on each ring BEFORE the smalls, so the PE's
            # first h matmul starts ~3us earlier (matters when the PE, not
            # the stream, is the binding path). Ring totals are unchanged.
            h_view = h_d[:].rearrange("(n p c) f -> n p (c f)", p=128, c=cr)
            early_ht = []
            for n in range(2):
                ht = hpool.tile([128, cr * F], hdt, tag="htile")
                (nc.sync if n == 0 else nc.scalar).dma_start(
                    out=ht[:], in_=h_view[n])
                early_ht.append(ht)

            # ---- small loads, balanced across the two HWDGE rings so both
            # rings finish together: sT/b + WT-half + hs[0:64] on sync
            # (0.649MB), WT-half + hs[64:128] on scalar (0.64MB). Lopsided
            # smalls push one ring's finish (and the stream end) out;
            # SWDGE (gpsimd) smalls throttle the shared SDMA engines
            # during the ramp.
            st_sb = small.tile([128, 2, BL], f32)
            nc.sync.dma_start(
                out=st_sb[:], in_=st_d[:].rearrange("(c p) b -> p c b", p=128))
            wt_sb = small.tile([128, 2, H], f32)
            wt_view = wt_d[:].rearrange("(c p) h -> p c h", p=128)
            nc.sync.dma_start(out=wt_sb[:, 0, :], in_=wt_view[:, 0, :])
            nc.scalar.dma_start(out=wt_sb[:, 1, :], in_=wt_view[:, 1, :])
            b_sb = small.tile([1, H], f32)
            nc.sync.dma_start(out=b_sb[:], in_=b_d[:])
            hs_sb = small.tile([N, F], f32)
            nc.sync.dma_start(out=hs_sb[0:64, :], in_=hs_d[:][0:64, :])
            nc.scalar.dma_start(out=hs_sb[64:128, :], in_=hs_d[:][64:128, :])

            # ---- constants (e3/e3r first: they gate the first h matmul) --
            # E3[p, c, m] = 1.0 iff m == c ; E3[:, b, :] is the ones-column
            # selector landing batch b's column sums on PSUM partition b.
            e3 = consts.tile([128, BL, BL], f32)
            nc.gpsimd.memset(e3[:], 0.0)
            nc.gpsimd.affine_select(
                out=e3[:], in_=e3[:], compare_op=AO.not_equal, fill=1.0,
                base=0, pattern=[[-1, BL], [1, BL]], channel_multiplier=0,
            )
            if hdt != f32:
                e3r = consts.tile([128, BL, BL], hdt)
                nc.vector.tensor_copy(out=e3r[:], in_=e3[:])
            else:
                e3r = e3
            ones1 = consts.tile([1, 128], f32)
            nc.gpsimd.memset(ones1[:], 1.0)
            ones8 = consts.tile([BL, 128], f32)
            nc.gpsimd.memset(ones8[:], 1.0)
            # ebd[b, b', h] = 1.0 iff b' == b  (block-diagonal placement mask)
            ebd = consts.tile([BL, BL, H], f32)
            nc.gpsimd.memset(ebd[:], 0.0)
            nc.gpsimd.affine_select(
                out=ebd[:], in_=ebd[:], compare_op=AO.not_equal, fill=1.0,
                base=0, pattern=[[-1, BL], [0, H]], channel_multiplier=1,
            )

            def sw_path():
                # sW = s @ W.T + b -> [BL, H] (batch on partitions), from
                # the host-transposed operands: out[b,h] = sum_d sT[d,b]
                # * WT[d,h]. Three matmuls, no on-chip transposes.
                ps_sw = psum.tile([BL, H], f32, tag="tmp")
                nc.tensor.matmul(ps_sw[:], st_sb[:, 0, :], wt_sb[:, 0, :],
                                 start=True, stop=False)
                nc.tensor.matmul(ps_sw[:], st_sb[:, 1, :], wt_sb[:, 1, :],
                                 start=False, stop=False)
                nc.tensor.matmul(ps_sw[:], ones1[0:1, 0:BL], b_sb[:],
                                 start=False, stop=True)
                sw_sb = small.tile([BL, H], f32)
                nc.vector.tensor_copy(out=sw_sb[:], in_=ps_sw[:])

                # sW placed block-diagonally: sw_bd[b, b', :] = sW[b]*[b'==b]
                # so ones8^T @ sw_bd broadcasts sW to all 128 partitions
                # with no DRAM bounce. NOTE: the whole scores path must
                # stay fp32 — f32r truncation here adds ADDITIVE noise to
                # c_t, blowing up max-rel error on near-zero outputs (the
                # h_sum f32r path only perturbs the per-batch scale).
                sw_bd = small.tile([BL, BL, H], f32)
                nc.vector.tensor_mul(
                    out=sw_bd[:],
                    in0=sw_sb[:].unsqueeze(1).to_broadcast((BL, BL, H)),
                    in1=ebd[:],
                )
                return sw_sb, sw_bd[:].rearrange("b a h -> b (a h)")

            def scores_part1(sw_bd_flat):
                # broadcast sW to all 128 partitions (PE)
                ps_bc = psum.tile([128, F], f32, tag="big4")
                for c in range(4):
                    nc.tensor.matmul(
                        ps_bc[:, c * 512:(c + 1) * 512],
                        ones8[:], sw_bd_flat[:, c * 512:(c + 1) * 512],
                        start=True, stop=True,
                    )
                # scores_raw[n, b] = sum_h sW[b,h] * hs[n,b,h]
                prod = small.tile([N, F], f32)
                nc.vector.tensor_mul(out=prod[:], in0=hs_sb[:], in1=ps_bc[:])
                scores = small.tile([N, BL], f32)
                nc.vector.reduce_sum(
                    out=scores[:],
                    in_=prod[:].rearrange("n (b h) -> n b h", b=BL), axis=X,
                )
                # scoresE[:, b, :] is scores[:, b] placed in column b, zeros
                # elsewhere, so each matmul only lands on PSUM partition b.
                scores_e = small.tile([N, BL, BL], f32)
                nc.vector.tensor_mul(
                    out=scores_e[:],
                    in0=scores[:].unsqueeze(2).to_broadcast((N, BL, BL)),
                    in1=e3[:],
                )
                return scores_e

            def scores_part2(scores_e):
                ps_o = psum.tile([BL, H], f32, tag="cout")
                for bb in range(BL):
                    nc.tensor.matmul(
                        ps_o[:], scores_e[:, bb, :],
                        hs_sb[:, bb * H:(bb + 1) * H],
                        start=(bb == 0), stop=(bb == BL - 1),
                        skip_group_check=True,
                    )
                return ps_o

            # ---- the big stream: h_sum over T as 1MB contiguous chunks.
            # Chunks 0..nch-2 accumulate into ps8; the last chunk into
            # ps8b, so the <sW, ps8> denominator partial runs on DVE while
            # the last chunk is still streaming — off the tail critical
            # path. After the final matmul only the small ps8b partial +
            # combine + reciprocal + scale remain.
            ps8 = psumb.tile([BL, H], f32)
            ps8b = psum.tile([BL, H], f32, tag="den_b")
            first_mm = True
            first_mm_b = True
            sw_sb = sw_bd_flat = None
            scores_e = None
            ps_o = None
            denq_a = small.tile([BL, H], f32)
            den_a = small.tile([BL, 1], f32)
            for n in range(nch - 1):
                if n < 2:
                    ht = early_ht[n]
                else:
                    ht = hpool.tile([128, cr * F], hdt, tag="htile")
                    dma_eng = nc.sync if n % 2 == 0 else nc.scalar
                    dma_eng.dma_start(out=ht[:], in_=h_view[n])
                for c in range(cr):
                    for bb in range(BL):
                        stop = (n == nch - 2 and c == cr - 1 and bb == BL - 1)
                        nc.tensor.matmul(
                            ps8[:], e3r[:, bb, :],
                            ht[:, c * F + bb * H: c * F + (bb + 1) * H],
                            start=first_mm, stop=stop,
                            skip_group_check=True,
                        )
                        first_mm = False
                if n == sw_at:
                    sw_sb, sw_bd_flat = sw_path()
                if n == sc1_at:
                    scores_e = scores_part1(sw_bd_flat)
                if n == sc2_at:
                    ps_o = scores_part2(scores_e)
                if n == nch - 2:
                    # denominator partial over chunks 0..nch-2 — overlaps
                    # the last chunk's DMA.
                    nc.vector.tensor_mul(out=denq_a[:], in0=sw_sb[:], in1=ps8[:])
                    nc.vector.reduce_sum(out=den_a[:], in_=denq_a[:], axis=X)

            # Last chunk as cr sub-chunks of one row per partition
            # ([128, F] contiguous, one per ring) so the final matmuls lag
            # the last byte by only 1/cr of a chunk.
            # (both on scalar: main loop gives sync 16 / scalar 15 chunks,
            # so the tail's cr sub-chunks even the rings out)
            hv1 = h_d[:].rearrange("(m p) f -> m p f", p=128)
            m0 = (nch - 1) * cr
            for k in range(cr):
                htl = hpool.tile([128, F], hdt, tag="htile")
                nc.scalar.dma_start(out=htl[:], in_=hv1[m0 + k])
                for bb in range(BL):
                    stop = (k == cr - 1 and bb == BL - 1)
                    nc.tensor.matmul(
                        ps8b[:], e3r[:, bb, :],
                        htl[:, bb * H: (bb + 1) * H],
                        start=first_mm_b, stop=stop,
                        skip_group_check=True,
                    )
                    first_mm_b = False

            # ---- last-chunk denom partial, combine, reciprocal, store ----
            denq_b = small.tile([BL, H], f32)
            den_b = small.tile([BL, 1], f32)
            nc.vector.tensor_mul(out=denq_b[:], in0=sw_sb[:], in1=ps8b[:])
            nc.vector.reduce_sum(out=den_b[:], in_=denq_b[:], axis=X)
            den = small.tile([BL, 1], f32)
            nc.vector.tensor_add(out=den[:], in0=den_a[:], in1=den_b[:])
            inv = small.tile([BL, 1], f32)
            nc.vector.reciprocal(out=inv[:], in_=den[:])
            c_fin = small.tile([BL, H], f32)
            nc.vector.tensor_scalar_mul(out=c_fin[:], in0=ps_o[:], scalar1=inv[:])
            nc.scalar.dma_start(out=out_d[:], in_=c_fin[:])

    _install_birpatch(nc)
    return nc


def _get_nc(**kw):
    key = tuple(sorted(kw.items()))
    if key not in _CACHE:
        _CACHE[key] = _build(**kw)
    return _CACHE[key]


def _np_hdt(hdt_name):
    if hdt_name == "f16":
        return np.float16
    if hdt_name == "bf16":
        import ml_dtypes
        return ml_dtypes.bfloat16
    return np.float32


def _shard_inputs(s_before, h_sliced, h, W, b, t_total=T, hdt_name="f16"):
    np_h = _np_hdt(hdt_name)
    in_maps = []
    for i in range(NCORES):
        sl = slice(i * BL, (i + 1) * BL)
        in_maps.append({
            "h": np.ascontiguousarray(
                h[:t_total, sl, :].astype(np_h)).reshape(t_total, F),
            "hs": np.ascontiguousarray(h_sliced[:, sl, :]).reshape(N, F),
            # host-transposed (d-major) so the PE contracts over d on the
            # partition dim with no on-chip transposes
            "st": np.ascontiguousarray(s_before[0, sl, :].T),
            "wt": np.ascontiguousarray(W.T),
            "bias": np.ascontiguousarray(b).reshape(1, H),
        })
    return in_maps


def _run(s_before, h_sliced, h, W, b, trace=False, **build_kw):
    from concourse.bass_utils import run_bass_kernel_spmd

    nc = _get_nc(**build_kw)
    in_maps = _shard_inputs(s_before, h_sliced, h, W, b,
                            t_total=build_kw.get("t_total", T),
                            hdt_name=build_kw.get("hdt_name", "f16"))
    bkr = run_bass_kernel_spmd(nc, in_maps, list(range(NCORES)), trace=trace)
    out = np.concatenate([bkr.results[i]["out"] for i in range(NCORES)], axis=0)
    return out, bkr


def kernel(s_before, h_sliced, h, W, b):
    out, _ = _run(
        np.asarray(s_before), np.asarray(h_sliced), np.asarray(h),
        np.asarray(W), np.asarray(b),
    )
    return out



# revision 29
# speedup vs baseline: 1.0048x; 1.0048x over previous
"""Trainium2 Bass kernel for nn_Attention_35708358099413.

Reference computation (T=8192, B=64, H=256, N=128):
    sW     = s_before @ W.T + b                      # [1,B,H]
    denom  = einsum('obd,tbd->ob', sW, h)            # [1,B] (sum over T and H)
    scores = einsum('obd,nbd->obn', sW, h_sliced) / denom
    c_t    = (scores.T * h_sliced).sum(0)            # [B,H]

Strategy: pure data-parallel over batch. 8 cores x 8 batches each; no
collectives. Per core the dominant work is h_sum[b,d] = sum_t h[t,b,d]
— a pure streaming reduction, so HBM read bytes are the binding
resource end to end.

KEY LEVER — h streams as fp16 (host-side cast in _shard_inputs):
32MB/core instead of 64MB. This is safe because h only enters the
output through denom[b] = <sW[b], h_sum[b]>: the quantization error is
MULTIPLICATIVE per batch (~5e-4 for fp16), nothing additive hits the
near-zero output elements. bf16 would NOT pass: the observed max-rel
comes from a batch with a small denominator, and scaling fp16's 3.5e-3
max-rel by the 8x eps ratio lands over the 2e-2 gate. The scores/c_t
path (h_sliced, s, W, b) stays fp32: any quantization there adds
ADDITIVE noise that blows up max-rel on near-zero outputs.

Under full 8-core overlap each core sustains ~317-330 GB/s of HBM
read (the ~358 GB/s HBM-per-NC share minus arbitration); measured
chunk-size sweep: 1MB chunks (8KB/partition contiguous descriptors,
cr=2) best; 2MB equal; 4MB and 0.5MB worse. h streams as 31 full
chunks alternating the two HWDGE rings (sync 16 / scalar 15) + the
last chunk as two 0.5MB [128, 2048] sub-chunks both on scalar (evens
the rings at ~16.65MB each AND halves the final matmul lag after the
last byte). The ~1.3MB of fp32 small loads are ring-balanced: s/b +
W-half + hs[0:64] on sync, W-half + hs[64:128] on scalar. Do NOT
split chunks into strided sub-1MB DMAs (4KB strided descriptors
collapse to a single-SDMA-engine trickle); the tail sub-chunks are
whole contiguous [128, F] blocks, which are fine.

Reduction over T on the TensorEngine as fp16 matmuls: lhsT =
e3[:, b, :] (ones in column b) lands batch b's column sums on PSUM
partition b. Chunks 0..30 accumulate into ps8; the last chunk into a
separate ps8b, so the <sW, ps8> denominator partial runs on DVE while
the last chunk is still streaming — after the final matmul only the
small ps8b partial + combine + reciprocal + scale + 8KB store remain
(~2.5us tail). Matmuls are emitted per-chunk (16 per 1MB chunk) so
the in-order PE tracks DMA arrivals; warm PE does N=256 matmuls in
~109ns, well ahead of the ~2.9us/chunk DMA cadence.

sW = s @ W.T + b on PE from on-chip transposes of s and W, emitted
after chunk 2; sW is broadcast to all 128 partitions by placing it
block-diagonally and multiplying by ones8 on PE — no DRAM bounce.
scores_raw[n,b] = rowwise reduce of (h_sliced * bcast_sW) on DVE;
c_raw[b,:] = scores^T @ h_sliced on PE via masked score columns.

Measured: 220us (f32r, 64MB) -> ~111us (fp16, 32MB) on 8-core SPMD;
breakdown ~7.7us NRT+Tile preamble, ~93-101us stream at the HBM wall,
~2.5us tail compute+store, ~8.7us NRT postamble + profile flush (the
pre/postamble are kernel-invariant: 51 sems/engine reset, barriers).
Rep jitter is bimodal (+0/+10us) from cross-core HBM contention phase.
"""

import json

import numpy as np

T, B, H, N = 8192, 64, 256, 128
NCORES = 8
BL = B // NCORES          # 8 batches per core
F = BL * H                # 2048

_CACHE = {}


def _split_multi_waits(bir_bytes, max_waits=1):
    """Walrus in some containers rejects instructions carrying more than
    one sem wait ("Too many sync wait commands"). Move excess waits onto
    preceding same-engine Drain carrier instructions."""
    m = json.loads(bir_bytes)
    for fn in m.get("functions", []):
        for bb in fn.get("blocks", []):
            out = []
            for inst in bb.get("instructions", []):
                si = inst.get("sync_info") or {}
                w = si.get("on_wait") or []
                if len(w) > max_waits:
                    head = w[: len(w) - max_waits]
                    si["on_wait"] = w[len(w) - max_waits:]
                    inst["sync_info"] = si
                    for k, wt in enumerate(head):
                        out.append({
                            "name": f"{inst['name']}_wsplit{k}",
                            "engine": inst["engine"],
                            "opcode": "Drain",
                            "ins": [], "outs": [],
                            "is_reset_sema": False,
                            "debug": inst.get("debug"),
                            "sync_info": {"on_wait": [wt], "on_update": []},
                        })
                out.append(inst)
            bb["instructions"] = out
    return json.dumps(m).encode()


def _install_birpatch(nc):
    orig = nc.to_json_bytes
    nc.to_json_bytes = lambda: _split_multi_waits(orig())


def _build(t_total=T, cr=2, hbufs=20, hdt_name="f16",
           sw_at=-1, sc1_at=-1, sc2_at=-1):
    """cr = h-rows per partition per chunk (chunk contiguous; for f16
    cr=2 keeps the proven 8KB/partition descriptor shape = 1MB chunk).
    hdt_name: dtype h is streamed in. 'f16' halves HBM bytes vs f32/f32r;
    the h path only perturbs each batch's denominator scale
    (multiplicative, ~5e-4 for fp16) so the rel-err gate is safe."""
    import concourse.bass as bass
    import concourse.mybir as mybir
    from concourse import tile

    f32 = mybir.dt.float32
    X = mybir.AxisListType.X
    AO = mybir.AluOpType

    rows_per_chunk = 128 * cr
    nch = t_total // rows_per_chunk
    assert nch * rows_per_chunk == t_total
    # The PE executes strictly in order: anything the sW/scores path
    # waits on stalls every later h-chunk matmul behind it. Place the
    # stages so their DVE/DMA dependencies are long since ready when the
    # PE reaches them: sW matmuls after chunk 4 (s/W/b land ~chunk 3),
    # scores_part1 after chunk 6 (hs lands ~chunk 5), scores_part2 after
    # chunk 20 (the ~5us DVE prod/reduce chain has ~14 chunks of slack).
    if sw_at < 0:
        sw_at = min(4, nch - 4)
    if sc1_at < 0:
        sc1_at = min(6, nch - 3)
    if sc2_at < 0:
        sc2_at = (5 * nch) // 8

    hdt = {"f16": mybir.dt.float16, "bf16": mybir.dt.bfloat16,
           "f32r": mybir.dt.float32r, "f32": f32}[hdt_name]

    nc = bass.Bass()
    h_d = nc.dram_tensor("h", [t_total, F], hdt, kind="ExternalInput")
    hs_d = nc.dram_tensor("hs", [N, F], f32, kind="ExternalInput")
    # s and W arrive HOST-TRANSPOSED (d-major): sW = s @ W.T contracts
    # over d, and the PE contracts over partitions — with d on the
    # partition dim both matmul operands come straight from DRAM, no
    # on-chip transposes (the old PE-transpose -> DVE-copy ping-pong
    # stalled every later h-chunk matmul behind it in program order).
    st_d = nc.dram_tensor("st", [H, BL], f32, kind="ExternalInput")
    wt_d = nc.dram_tensor("wt", [H, H], f32, kind="ExternalInput")
    b_d = nc.dram_tensor("bias", [1, H], f32, kind="ExternalInput")
    out_d = nc.dram_tensor("out", [BL, H], f32, kind="ExternalOutput")

    with tile.TileContext(nc) as tc:
        with (
            tc.tile_pool(name="consts", bufs=1) as consts,
            tc.tile_pool(name="small", bufs=1) as small,
            tc.tile_pool(name="hpool", bufs=hbufs) as hpool,
            tc.tile_pool(name="psum", bufs=1, space=bass.MemorySpace.PSUM) as psum,
            tc.tile_pool(name="psumb", bufs=1, space=bass.MemorySpace.PSUM) as psumb,
        ):
            # ---- first chunk on each ring BEFORE the smalls, so the PE's
            # first h matmul starts ~3us earlier (matters when the PE, not
            # the stream, is the binding path). Ring totals are unchanged.
            # (Keep these the same [128, cr*F] geometry as the main loop:
            # allocating smaller tiles first from hpool raced once with a
            # NaN result — don't mix sizes at the pool head.)
            h_view = h_d[:].rearrange("(n p c) f -> n p (c f)", p=128, c=cr)
            hv1 = h_d[:].rearrange("(m p) f -> m p f", p=128)
            early_ht = []
            for n in range(2):
                ht = hpool.tile([128, cr * F], hdt, tag="htile")
                (nc.sync if n == 0 else nc.scalar).dma_start(
                    out=ht[:], in_=h_view[n])
                early_ht.append(ht)

            # ---- small loads, balanced across the two HWDGE rings so both
            # rings finish together: sT/b + WT-half + hs[0:64] on sync
            # (0.649MB), WT-half + hs[64:128] on scalar (0.64MB). Lopsided
            # smalls push one ring's finish (and the stream end) out;
            # SWDGE (gpsimd) smalls throttle the shared SDMA engines
            # during the ramp.
            st_sb = small.tile([128, 2, BL], f32)
            nc.sync.dma_start(
                out=st_sb[:], in_=st_d[:].rearrange("(c p) b -> p c b", p=128))
            wt_sb = small.tile([128, 2, H], f32)
            wt_view = wt_d[:].rearrange("(c p) h -> p c h", p=128)
            nc.sync.dma_start(out=wt_sb[:, 0, :], in_=wt_view[:, 0, :])
            nc.scalar.dma_start(out=wt_sb[:, 1, :], in_=wt_view[:, 1, :])
            b_sb = small.tile([1, H], f32)
            nc.sync.dma_start(out=b_sb[:], in_=b_d[:])
            hs_sb = small.tile([N, F], f32)
            nc.sync.dma_start(out=hs_sb[0:64, :], in_=hs_d[:][0:64, :])
            nc.scalar.dma_start(out=hs_sb[64:128, :], in_=hs_d[:][64:128, :])

            # ---- constants (e3/e3r first: they gate the first h matmul) --
            # E3[p, c, m] = 1.0 iff m == c ; E3[:, b, :] is the ones-column
            # selector landing batch b's column sums on PSUM partition b.
            e3 = consts.tile([128, BL, BL], f32)
            nc.gpsimd.memset(e3[:], 0.0)
            nc.gpsimd.affine_select(
                out=e3[:], in_=e3[:], compare_op=AO.not_equal, fill=1.0,
                base=0, pattern=[[-1, BL], [1, BL]], channel_multiplier=0,
            )
            if hdt != f32:
                e3r = consts.tile([128, BL, BL], hdt)
                nc.vector.tensor_copy(out=e3r[:], in_=e3[:])
            else:
                e3r = e3
            ones1 = consts.tile([1, 128], f32)
            nc.gpsimd.memset(ones1[:], 1.0)
            ones8 = consts.tile([BL, 128], f32)
            nc.gpsimd.memset(ones8[:], 1.0)
            # ebd[b, b', h] = 1.0 iff b' == b  (block-diagonal placement mask)
            ebd = consts.tile([BL, BL, H], f32)
            nc.gpsimd.memset(ebd[:], 0.0)
            nc.gpsimd.affine_select(
                out=ebd[:], in_=ebd[:], compare_op=AO.not_equal, fill=1.0,
                base=0, pattern=[[-1, BL], [0, H]], channel_multiplier=1,
            )

            def sw_path():
                # sW = s @ W.T + b -> [BL, H] (batch on partitions), from
                # the host-transposed operands: out[b,h] = sum_d sT[d,b]
                # * WT[d,h]. Three matmuls, no on-chip transposes.
                ps_sw = psum.tile([BL, H], f32, tag="tmp")
                nc.tensor.matmul(ps_sw[:], st_sb[:, 0, :], wt_sb[:, 0, :],
                                 start=True, stop=False)
                nc.tensor.matmul(ps_sw[:], st_sb[:, 1, :], wt_sb[:, 1, :],
                                 start=False, stop=False)
                nc.tensor.matmul(ps_sw[:], ones1[0:1, 0:BL], b_sb[:],
                                 start=False, stop=True)
                sw_sb = small.tile([BL, H], f32)
                nc.vector.tensor_copy(out=sw_sb[:], in_=ps_sw[:])

                # sW placed block-diagonally: sw_bd[b, b', :] = sW[b]*[b'==b]
                # so ones8^T @ sw_bd broadcasts sW to all 128 partitions
                # with no DRAM bounce. NOTE: the whole scores path must
                # stay fp32 — f32r truncation here adds ADDITIVE noise to
                # c_t, blowing up max-rel error on near-zero outputs (the
                # h_sum f32r path only perturbs the per-batch scale).
                sw_bd = small.tile([BL, BL, H], f32)
                nc.vector.tensor_mul(
                    out=sw_bd[:],
                    in0=sw_sb[:].unsqueeze(1).to_broadcast((BL, BL, H)),
                    in1=ebd[:],
                )
                return sw_sb, sw_bd[:].rearrange("b a h -> b (a h)")

            def scores_part1(sw_bd_flat):
                # broadcast sW to all 128 partitions (PE)
                ps_bc = psum.tile([128, F], f32, tag="big4")
                for c in range(4):
                    nc.tensor.matmul(
                        ps_bc[:, c * 512:(c + 1) * 512],
                        ones8[:], sw_bd_flat[:, c * 512:(c + 1) * 512],
                        start=True, stop=True,
                    )
                # scores_raw[n, b] = sum_h sW[b,h] * hs[n,b,h]
                prod = small.tile([N, F], f32)
                nc.vector.tensor_mul(out=prod[:], in0=hs_sb[:], in1=ps_bc[:])
                scores = small.tile([N, BL], f32)
                nc.vector.reduce_sum(
                    out=scores[:],
                    in_=prod[:].rearrange("n (b h) -> n b h", b=BL), axis=X,
                )
                # scoresE[:, b, :] is scores[:, b] placed in column b, zeros
                # elsewhere, so each matmul only lands on PSUM partition b.
                scores_e = small.tile([N, BL, BL], f32)
                nc.vector.tensor_mul(
                    out=scores_e[:],
                    in0=scores[:].unsqueeze(2).to_broadcast((N, BL, BL)),
                    in1=e3[:],
                )
                return scores_e

            def scores_part2(scores_e):
                ps_o = psum.tile([BL, H], f32, tag="cout")
                for bb in range(BL):
                    nc.tensor.matmul(
                        ps_o[:], scores_e[:, bb, :],
                        hs_sb[:, bb * H:(bb + 1) * H],
                        start=(bb == 0), stop=(bb == BL - 1),
                        skip_group_check=True,
                    )
                return ps_o

            # ---- the big stream: h_sum over T as 1MB contiguous chunks.
            # Chunks 0..nch-2 accumulate into ps8; the last chunk into
            # ps8b, so the <sW, ps8> denominator partial runs on DVE while
            # the last chunk is still streaming — off the tail critical
            # path. After the final matmul only the small ps8b partial +
            # combine + reciprocal + scale remain.
            ps8 = psumb.tile([BL, H], f32)
            ps8b = psum.tile([BL, H], f32, tag="den_b")
            first_mm = True
            first_mm_b = True
            sw_sb = sw_bd_flat = None
            scores_e = None
            ps_o = None
            denq_a = small.tile([BL, H], f32)
            den_a = small.tile([BL, 1], f32)
            for n in range(nch - 1):
                if n < 2:
                    ht = early_ht[n]
                else:
                    ht = hpool.tile([128, cr * F], hdt, tag="htile")
                    dma_eng = nc.sync if n % 2 == 0 else nc.scalar
                    dma_eng.dma_start(out=ht[:], in_=h_view[n])
                for c in range(cr):
                    for bb in range(BL):
                        stop = (n == nch - 2 and c == cr - 1 and bb == BL - 1)
                        nc.tensor.matmul(
                            ps8[:], e3r[:, bb, :],
                            ht[:, c * F + bb * H: c * F + (bb + 1) * H],
                            start=first_mm, stop=stop,
                            skip_group_check=True,
                        )
                        first_mm = False
                if n == sw_at:
                    sw_sb, sw_bd_flat = sw_path()
                if n == sc1_at:
                    scores_e = scores_part1(sw_bd_flat)
                if n == sc2_at:
                    ps_o = scores_part2(scores_e)
                if n == nch - 2:
                    # denominator partial over chunks 0..nch-2 — overlaps
                    # the last chunk's DMA.
                    nc.vector.tensor_mul(out=denq_a[:], in0=sw_sb[:], in1=ps8[:])
                    nc.vector.reduce_sum(out=den_a[:], in_=denq_a[:], axis=X)

            # Last chunk as cr sub-chunks of one row per partition
            # ([128, F] contiguous, one per ring) so the final matmuls lag
            # the last byte by only 1/cr of a chunk.
            # (both on scalar: with the early/main split above, sync
            # carries 1MB early + 15 main chunks + 0.649MB smalls and
            # scalar 1MB early + 14 main + 1MB tail + 0.64MB smalls)
            m0 = (nch - 1) * cr
            for k in range(cr):
                htl = hpool.tile([128, F], hdt, tag="htile")
                nc.scalar.dma_start(out=htl[:], in_=hv1[m0 + k])
                for bb in range(BL):
                    stop = (k == cr - 1 and bb == BL - 1)
                    nc.tensor.matmul(
                        ps8b[:], e3r[:, bb, :],
                        htl[:, bb * H: (bb + 1) * H],
                        start=first_mm_b, stop=stop,
                        skip_group_check=True,
                    )
                    first_mm_b = False

            # ---- last-chunk denom partial, combine, reciprocal, store ----
            denq_b = small.tile([BL, H], f32)
            den_b = small.tile([BL, 1], f32)
            nc.vector.tensor_mul(out=denq_b[:], in0=sw_sb[:], in1=ps8b[:])
            nc.vector.reduce_sum(out=den_b[:], in_=denq_b[:], axis=X)
            den = small.tile([BL, 1], f32)
            nc.vector.tensor_add(out=den[:], in0=den_a[:], in1=den_b[:])
            inv = small.tile([BL, 1], f32)
            nc.vector.reciprocal(out=inv[:], in_=den[:])
            c_fin = small.tile([BL, H], f32)
            nc.vector.tensor_scalar_mul(out=c_fin[:], in0=ps_o[:], scalar1=inv[:])
            nc.scalar.dma_start(out=out_d[:], in_=c_fin[:])

    _install_birpatch(nc)
    return nc


def _get_nc(**kw):
    key = tuple(sorted(kw.items()))
    if key not in _CACHE:
        _CACHE[key] = _build(**kw)
    return _CACHE[key]


def _np_hdt(hdt_name):
    if hdt_name == "f16":
        return np.float16
    if hdt_name == "bf16":
        import ml_dtypes
        return ml_dtypes.bfloat16
    return np.float32


def _shard_inputs(s_before, h_sliced, h, W, b, t_total=T, hdt_name="f16"):
    np_h = _np_hdt(hdt_name)
    in_maps = []
    for i in range(NCORES):
        sl = slice(i * BL, (i + 1) * BL)
        in_maps.append({
            "h": np.ascontiguousarray(
                h[:t_total, sl, :].astype(np_h)).reshape(t_total, F),
            "hs": np.ascontiguousarray(h_sliced[:, sl, :]).reshape(N, F),
            # host-transposed (d-major) so the PE contracts over d on the
            # partition dim with no on-chip transposes
            "st": np.ascontiguousarray(s_before[0, sl, :].T),
            "wt": np.ascontiguousarray(W.T),
            "bias": np.ascontiguousarray(b).reshape(1, H),
        })
    return in_maps


def _run(s_before, h_sliced, h, W, b, trace=False, **build_kw):
    from concourse.bass_utils import run_bass_kernel_spmd

    nc = _get_nc(**build_kw)
    in_maps = _shard_inputs(s_before, h_sliced, h, W, b,
                            t_total=build_kw.get("t_total", T),
                            hdt_name=build_kw.get("hdt_name", "f16"))
    bkr = run_bass_kernel_spmd(nc, in_maps, list(range(NCORES)), trace=trace)
    out = np.concatenate([bkr.results[i]["out"] for i in range(NCORES)], axis=0)
    return out, bkr


def kernel(s_before, h_sliced, h, W, b):
    out, _ = _run(
        np.asarray(s_before), np.asarray(h_sliced), np.asarray(h),
        np.asarray(W), np.asarray(b),
    )
    return out

